# revision 17
# baseline (speedup 1.0000x reference)
"""CRF negative-log-likelihood loss on 8 TRN2 NeuronCores.

Strategy (pure data parallel per sharding hint): batch dim (256) sharded
32/core. Each core runs the forward algorithm (denominator) in the exp
domain: state P[j,b] = exp(score[j,b] - c[b] - t*ALPHA), stepped as
P <- (exp(trans)^T @ P) * exp(e_t - ALPHA), with a per-batch sum
renormalization every NORM_EVERY steps (log z accumulated into c).
The gold-path numerator is a tiny gather (B*S lookups) done on host.
"""

import sys

import numpy as np

for _p in ("/opt/trn_rl_repo",):
    if _p not in sys.path:
        sys.path.insert(0, _p)

B, S, T = 256, 2048, 48
NCORES = 8
BC = B // NCORES  # 32 batches per core
CHUNK = 128
NCHUNK = S // CHUNK
ALPHA = 4.4  # mean per-step log growth, folded into the emission exp
NORM_EVERY = 64

_CACHE = {}


def _split_multi_waits(nc, mybir):
    """HW allows one semaphore wait per instruction; move extras onto
    same-engine NoOps inserted just before (what Bacc's
    generate_event_semaphores does, minus the EventSemaphore encoding
    this walrus build rejects)."""
    k = 0
    for f in nc.m.functions:
        for blk in f.blocks:
            out = []
            for inst in blk.instructions:
                si = inst.sync_info
                if si is not None and si.on_wait and len(si.on_wait) > 1:
                    waits = list(si.on_wait)
                    for w in waits[:-1]:
                        k += 1
                        out.append(
                            mybir.InstNoOp(
                                name=f"splitw-{k}",
                                sync_info=mybir.SyncInfo(
                                    on_wait=[w], on_update=[]
                                ),
                                engine=inst.engine,
                                bass_nofuse=True,
                            )
                        )
                    inst.sync_info = mybir.SyncInfo(
                        on_wait=[waits[-1]], on_update=list(si.on_update)
                    )
                out.append(inst)
            blk.instructions[:] = out


def _build():
    import concourse.bass as bass
    import concourse.mybir as mybir
    from concourse.tile import TileContext

    AF = mybir.ActivationFunctionType
    f32 = mybir.dt.float32

    nc = bass.Bass()
    em = nc.declare_dram_parameter("emissions", [BC, S, T], f32, isOutput=False)
    tr = nc.declare_dram_parameter("transitions", [T, T], f32, isOutput=False)
    id_p = nc.declare_dram_parameter("ident", [CHUNK, CHUNK], f32, isOutput=False)
    out = nc.declare_dram_parameter("out", [1, BC], f32, isOutput=True)

    with TileContext(nc) as tc:
        with (
            tc.tile_pool(name="const", bufs=1) as constp,
            tc.tile_pool(name="stage", bufs=6) as stagep,
            tc.tile_pool(name="fc", bufs=2) as fcp,
            tc.tile_pool(name="state", bufs=2) as statep,
            tc.tile_pool(name="acc", bufs=1) as accp,
            tc.tile_pool(name="nrm", bufs=2) as nrmp,
            tc.tile_pool(name="psq", bufs=2, space="PSUM") as psq,
            tc.tile_pool(name="pst", bufs=4, space="PSUM") as pst,
            tc.tile_pool(name="psn", bufs=1, space="PSUM") as psn,
        ):
            # constants
            zconst = constp.tile([128, 1], f32)
            nc.vector.memset(zconst[:], 0.0)
            nc.const_aps.aps[(f32, 0.0)] = zconst[:]
            nbias = constp.tile([128, 1], f32)
            nc.vector.memset(nbias[:], -ALPHA)
            traw = constp.tile([T, T], f32)
            nc.sync.dma_start(out=traw[:], in_=tr[:])
            E = constp.tile([T, T], f32)
            nc.scalar.activation(E[:], traw[:], AF.Exp)  # exp(transitions)
            ident = constp.tile([CHUNK, CHUNK], f32)
            nc.sync.dma_start(out=ident[:], in_=id_p[:])
            ones_col = constp.tile([T, 1], f32)
            nc.vector.memset(ones_col[:], 1.0)
            ones_row = constp.tile([1, T], f32)
            nc.vector.memset(ones_row[:], 1.0)
            c_acc = accp.tile([1, BC], f32)
            nc.vector.memset(c_acc[:], 0.0)

            p_cur = None
            for ch in range(NCHUNK):
                t0 = ch * CHUNK
                fc = fcp.tile([T, BC, CHUNK], f32)
                for b in range(BC):
                    stage = stagep.tile([CHUNK, T], f32, tag="stage")
                    nc.sync.dma_start(
                        out=stage[:], in_=em[b, t0 : t0 + CHUNK, :]
                    )
                    pt = pst.tile([T, CHUNK], f32)
                    nc.tensor.transpose(pt[:], stage[:], ident[:])
                    nc.scalar.activation(
                        out=fc[:, b, :], in_=pt[:], func=AF.Exp, bias=nbias[:T]
                    )
                for t in range(CHUNK):
                    gt = t0 + t
                    ft = fc[:, :, t]  # [T, BC] view, stride CHUNK
                    if gt == 0:
                        p_new = statep.tile([T, BC], f32, tag="p")
                        nc.vector.tensor_copy(out=p_new[:], in_=ft)
                        p_cur = p_new
                        continue
                    q = psq.tile([T, BC], f32)
                    nc.tensor.matmul(q[:], E[:], p_cur[:], start=True, stop=True)
                    if gt % NORM_EVERY == 0:
                        r = statep.tile([T, BC], f32, tag="r")
                        nc.vector.tensor_mul(out=r[:], in0=q[:], in1=ft)
                        z = psn.tile([1, BC], f32)
                        nc.tensor.matmul(
                            z[:], ones_col[:], r[:], start=True, stop=True
                        )
                        logz = nrmp.tile([1, BC], f32)
                        nc.scalar.activation(logz[:], z[:], AF.Ln)
                        nc.vector.tensor_add(
                            out=c_acc[:], in0=c_acc[:], in1=logz[:]
                        )
                        rz = nrmp.tile([1, BC], f32)
                        nc.vector.reciprocal(rz[:], z[:])
                        zb = psn.tile([T, BC], f32)
                        nc.tensor.matmul(
                            zb[:], ones_row[:], rz[:], start=True, stop=True
                        )
                        p_new = statep.tile([T, BC], f32, tag="p")
                        nc.vector.tensor_mul(out=p_new[:], in0=r[:], in1=zb[:])
                    else:
                        p_new = statep.tile([T, BC], f32, tag="p")
                        nc.vector.tensor_mul(out=p_new[:], in0=q[:], in1=ft)
                    p_cur = p_new

            zf = psn.tile([1, BC], f32, tag="z")
            nc.tensor.matmul(zf[:], ones_col[:], p_cur[:], start=True, stop=True)
            logzf = nrmp.tile([1, BC], f32)
            nc.scalar.activation(logzf[:], zf[:], AF.Ln)
            nc.vector.tensor_add(out=c_acc[:], in0=c_acc[:], in1=logzf[:])
            nc.sync.dma_start(out=out[:], in_=c_acc[:])

    _split_multi_waits(nc, mybir)
    return nc


def _get_nc():
    if "nc" not in _CACHE:
        _CACHE["nc"] = _build()
    return _CACHE["nc"]


def kernel(emissions, tags, mask, transitions):
    from concourse.bass_utils import run_bass_kernel_spmd

    emissions = np.ascontiguousarray(np.asarray(emissions, dtype=np.float32))
    tags = np.asarray(tags)
    mask = np.asarray(mask)
    transitions = np.ascontiguousarray(np.asarray(transitions, dtype=np.float32))

    # --- numerator: gold path score (tiny gather, host) ---
    maskf = mask.astype(np.float32)
    emit = np.take_along_axis(emissions, tags[:, :, None].astype(np.int64), axis=2)[
        ..., 0
    ]
    trans_path = transitions[tags[:, :-1], tags[:, 1:]]
    numerator = emit[:, 0] + ((trans_path + emit[:, 1:]) * maskf[:, 1:]).sum(axis=1)

    # --- denominator: forward algorithm on 8 NeuronCores ---
    nc = _get_nc()
    in_maps = [
        {
            "emissions": np.ascontiguousarray(
                emissions[c * BC : (c + 1) * BC]
            ),
            "transitions": transitions,
            "ident": np.eye(CHUNK, dtype=np.float32),
        }
        for c in range(NCORES)
    ]
    res = run_bass_kernel_spmd(nc, in_maps, core_ids=list(range(NCORES)))
    den = np.concatenate([res.results[c]["out"][0] for c in range(NCORES)])
    den = den + np.float32(S * ALPHA)

    llh = (numerator - den).mean()
    return np.asarray(llh, dtype=np.float32)


# revision 18
# speedup vs baseline: 1.1331x; 1.1331x over previous
"""CRF negative-log-likelihood loss on 8 TRN2 NeuronCores.

Strategy (pure data parallel per sharding hint): batch dim (256) sharded
32/core. Each core runs the forward algorithm (denominator) in the exp
domain: state P[j,b] = exp(score[j,b] - c[b] - t*ALPHA), stepped as
P <- (exp(trans)^T @ P) * exp(e_t - ALPHA), with a per-batch sum
renormalization every NORM_EVERY steps (log z accumulated into c).
The gold-path numerator is a tiny gather (B*S lookups) done on host.
"""

import sys

import numpy as np

for _p in ("/opt/trn_rl_repo", "/root/.axon_site/_ro/trn_rl_repo"):
    if _p not in sys.path:
        sys.path.insert(0, _p)

B, S, T = 256, 2048, 48
NCORES = 8
BC = B // NCORES  # 32 batches per core
CHUNK = 128
NCHUNK = S // CHUNK
ALPHA = 4.4  # mean per-step log growth, folded into the emission exp
NORM_EVERY = 64

_CACHE = {}


def _split_multi_waits(nc, mybir):
    """HW allows one semaphore wait per instruction; move extras onto
    same-engine NoOps inserted just before (what Bacc's
    generate_event_semaphores does, minus the EventSemaphore encoding
    this walrus build rejects)."""
    k = 0
    for f in nc.m.functions:
        for blk in f.blocks:
            out = []
            for inst in blk.instructions:
                si = inst.sync_info
                if si is not None and si.on_wait and len(si.on_wait) > 1:
                    waits = list(si.on_wait)
                    for w in waits[:-1]:
                        k += 1
                        out.append(
                            mybir.InstNoOp(
                                name=f"splitw-{k}",
                                sync_info=mybir.SyncInfo(
                                    on_wait=[w], on_update=[]
                                ),
                                engine=inst.engine,
                                bass_nofuse=True,
                            )
                        )
                    inst.sync_info = mybir.SyncInfo(
                        on_wait=[waits[-1]], on_update=list(si.on_update)
                    )
                out.append(inst)
            blk.instructions[:] = out


def _build():
    import concourse.bass as bass
    import concourse.mybir as mybir
    from concourse.tile import TileContext

    AF = mybir.ActivationFunctionType
    f32 = mybir.dt.float32

    nc = bass.Bass()
    em = nc.declare_dram_parameter("emissions", [BC, S, T], f32, isOutput=False)
    tr = nc.declare_dram_parameter("transitions", [T, T], f32, isOutput=False)
    id_p = nc.declare_dram_parameter("ident", [CHUNK, CHUNK], f32, isOutput=False)
    out = nc.declare_dram_parameter("out", [1, BC], f32, isOutput=True)

    with TileContext(nc) as tc:
        with (
            tc.tile_pool(name="const", bufs=1) as constp,
            tc.tile_pool(name="stage", bufs=6) as stagep,
            tc.tile_pool(name="fc", bufs=2) as fcp,
            tc.tile_pool(name="state", bufs=2) as statep,
            tc.tile_pool(name="acc", bufs=1) as accp,
            tc.tile_pool(name="nrm", bufs=2) as nrmp,
            tc.tile_pool(name="psq", bufs=2, space="PSUM") as psq,
            tc.tile_pool(name="pst", bufs=4, space="PSUM") as pst,
            tc.tile_pool(name="psn", bufs=1, space="PSUM") as psn,
        ):
            # constants
            zconst = constp.tile([128, 1], f32)
            nc.vector.memset(zconst[:], 0.0)
            nc.const_aps.aps[(f32, 0.0)] = zconst[:]
            nbias = constp.tile([128, 1], f32)
            nc.vector.memset(nbias[:], -ALPHA)
            traw = constp.tile([T, T], f32)
            nc.sync.dma_start(out=traw[:], in_=tr[:])
            E = constp.tile([T, T], f32)
            nc.scalar.activation(E[:], traw[:], AF.Exp)  # exp(transitions)
            ident = constp.tile([CHUNK, CHUNK], f32)
            nc.sync.dma_start(out=ident[:], in_=id_p[:])
            ones_col = constp.tile([T, 1], f32)
            nc.vector.memset(ones_col[:], 1.0)
            ones_row = constp.tile([1, T], f32)
            nc.vector.memset(ones_row[:], 1.0)
            c_acc = accp.tile([1, BC], f32)
            nc.vector.memset(c_acc[:], 0.0)

            p_cur = None
            for ch in range(NCHUNK):
                t0 = ch * CHUNK
                fc = fcp.tile([T, BC, CHUNK], f32)
                for b in range(BC):
                    stage = stagep.tile([CHUNK, T], f32, tag="stage")
                    nc.sync.dma_start(
                        out=stage[:], in_=em[b, t0 : t0 + CHUNK, :]
                    )
                    pt = pst.tile([T, CHUNK], f32)
                    nc.tensor.transpose(pt[:], stage[:], ident[:])
                    nc.scalar.activation(
                        out=fc[:, b, :], in_=pt[:], func=AF.Exp, bias=nbias[:T]
                    )
                for t in range(CHUNK):
                    gt = t0 + t
                    ft = fc[:, :, t]  # [T, BC] view, stride CHUNK
                    if gt == 0:
                        p_new = statep.tile([T, BC], f32, tag="p")
                        nc.vector.tensor_copy(out=p_new[:], in_=ft)
                        p_cur = p_new
                        continue
                    q = psq.tile([T, BC], f32)
                    nc.tensor.matmul(q[:], E[:], p_cur[:], start=True, stop=True)
                    if gt % NORM_EVERY == 0:
                        r = statep.tile([T, BC], f32, tag="r")
                        nc.vector.tensor_mul(out=r[:], in0=q[:], in1=ft)
                        z = psn.tile([1, BC], f32)
                        nc.tensor.matmul(
                            z[:], ones_col[:], r[:], start=True, stop=True
                        )
                        logz = nrmp.tile([1, BC], f32)
                        nc.scalar.activation(logz[:], z[:], AF.Ln)
                        nc.vector.tensor_add(
                            out=c_acc[:], in0=c_acc[:], in1=logz[:]
                        )
                        rz = nrmp.tile([1, BC], f32)
                        nc.vector.reciprocal(rz[:], z[:])
                        zb = psn.tile([T, BC], f32)
                        nc.tensor.matmul(
                            zb[:], ones_row[:], rz[:], start=True, stop=True
                        )
                        p_new = statep.tile([T, BC], f32, tag="p")
                        nc.vector.tensor_mul(out=p_new[:], in0=r[:], in1=zb[:])
                    else:
                        p_new = statep.tile([T, BC], f32, tag="p")
                        nc.vector.tensor_mul(out=p_new[:], in0=q[:], in1=ft)
                    p_cur = p_new

            zf = psn.tile([1, BC], f32, tag="z")
            nc.tensor.matmul(zf[:], ones_col[:], p_cur[:], start=True, stop=True)
            logzf = nrmp.tile([1, BC], f32)
            nc.scalar.activation(logzf[:], zf[:], AF.Ln)
            nc.vector.tensor_add(out=c_acc[:], in0=c_acc[:], in1=logzf[:])
            nc.sync.dma_start(out=out[:], in_=c_acc[:])

    _split_multi_waits(nc, mybir)
    return nc


def _get_nc():
    if "nc" not in _CACHE:
        _CACHE["nc"] = _build()
    return _CACHE["nc"]


def kernel(emissions, tags, mask, transitions):
    from concourse.bass_utils import run_bass_kernel_spmd

    emissions = np.ascontiguousarray(np.asarray(emissions, dtype=np.float32))
    tags = np.asarray(tags)
    mask = np.asarray(mask)
    transitions = np.ascontiguousarray(np.asarray(transitions, dtype=np.float32))

    # --- numerator: gold path score (tiny gather, host) ---
    maskf = mask.astype(np.float32)
    emit = np.take_along_axis(emissions, tags[:, :, None].astype(np.int64), axis=2)[
        ..., 0
    ]
    trans_path = transitions[tags[:, :-1], tags[:, 1:]]
    numerator = emit[:, 0] + ((trans_path + emit[:, 1:]) * maskf[:, 1:]).sum(axis=1)

    # --- denominator: forward algorithm on 8 NeuronCores ---
    nc = _get_nc()
    in_maps = [
        {
            "emissions": np.ascontiguousarray(
                emissions[c * BC : (c + 1) * BC]
            ),
            "transitions": transitions,
            "ident": np.eye(CHUNK, dtype=np.float32),
        }
        for c in range(NCORES)
    ]
    res = run_bass_kernel_spmd(nc, in_maps, core_ids=list(range(NCORES)))
    den = np.concatenate([res.results[c]["out"][0] for c in range(NCORES)])
    den = den + np.float32(S * ALPHA)

    llh = (numerator - den).mean()
    return np.asarray(llh, dtype=np.float32)
